# revision 3
# baseline (speedup 1.0000x reference)
"""Trainium2 Bass kernel for Kandinsky5 Nabla (block-sparse) attention.

Problem shape: B=2, S=2048, H=16, D=64, BLK=64 (32x32 block grid), P=0.9.

Sharding: 8 cores, each core owns 4 fully-independent (b, h) pairs
(b = core//4, heads 4*(core%4)..+4).  Host only slices inputs per core and
re-assembles the output; all compute (block-mask construction + masked SDPA)
runs on device.

Device algorithm per (b, h) pair:
  1. qT/kT [d, s] built via PE transposes of the natural [s, d] tiles.
  2. Block-pooled qaT/kaT [d, 32] via matmul against a constant block
     indicator (E2), m = qa @ ka^T.
  3. Sort-free nabla threshold: e = exp(m/8); keep[i,j] iff
     sum_{j': e_ij' <= e_ij} e_ij' >= 0.1 * sum_j' e_ij'   (equivalent to the
     reference's ascending-sort + cumsum >= 1-P rule, scale-invariant so no
     softmax normalization needed).  OR with sta (as multiplier (1-sta)).
  4. Mask folded into QK^T as K-augmentation: lhsT [96,128] = [kT ; A],
     rhs [96,512] = [qT ; C2] where A is the constant j->block indicator and
     C2[r, i] = NEG * (1 - keep[ib(i), r]) expanded along i.  A single f32r
     matmul then yields S^T[j, i] = k_j . q_i + NEG*(1-keep) directly.
  5. P^T = exp(S^T/8) on ACT (unstabilized: |s|<~7 so exp is safe; masked
     blocks get exp(-3750) == 0).
  6. PV with ones-augmented V (lhsT [128, 65]) accumulates O^T (rows 0..63)
     and the softmax denominator (row 64) in one PSUM accumulation.
  7. PE-transpose of [65, 128] chunks -> [128, 65], reciprocal of den column,
     scale rows, DMA out.
"""

import sys

for _p in ("/opt/trn_rl_repo", "/root/.axon_site/_ro/trn_rl_repo"):
    if _p not in sys.path:
        sys.path.append(_p)

from contextlib import ExitStack

import numpy as np

import concourse.bass as bass
import concourse.tile as tile
from concourse import bacc, mybir
from concourse.bass_utils import run_bass_kernel_spmd

F32 = mybir.dt.float32
F32R = mybir.dt.float32r
AF = mybir.ActivationFunctionType
ALU = mybir.AluOpType

B, S, H, D = 2, 2048, 16, 64
BLK = 64
S1 = S // BLK          # 32 blocks
NP = 4                 # (b,h) pairs per core
NT = S // 128          # 16 j-tiles of 128
NEG = -30000.0         # pre-scale additive mask value (exp(0.125*-30000) == 0)
THR = 0.1              # 1 - P
SCALE = 0.125          # 1/sqrt(D)

_CACHE = {}


def _build_program():
    nc = bacc.Bacc("TRN2", target_bir_lowering=False, debug=False, num_devices=8)

    q_in = nc.dram_tensor("q", [NP, S, D], F32, kind="ExternalInput").ap()
    k_in = nc.dram_tensor("k", [NP, S, D], F32, kind="ExternalInput").ap()
    v_in = nc.dram_tensor("v", [NP, S, D + 1], F32R, kind="ExternalInput").ap()
    e2_in = nc.dram_tensor("e2c", [128, 2], F32, kind="ExternalInput").ap()
    a_in = nc.dram_tensor("ac", [S1, S], F32R, kind="ExternalInput").ap()
    oms_in = nc.dram_tensor("oms", [S1, S1], F32, kind="ExternalInput").ap()
    id_in = nc.dram_tensor("idn", [128, 128], F32, kind="ExternalInput").ap()
    o_out = nc.dram_tensor("o", [NP, S, D], F32, kind="ExternalOutput").ap()

    with tile.TileContext(nc) as tc, ExitStack() as ctx:
        cpool = ctx.enter_context(tc.tile_pool(name="const", bufs=1))
        ident = cpool.tile([128, 128], F32)
        nc.sync.dma_start(ident[:], id_in)
        e2 = cpool.tile([128, 2], F32)
        nc.sync.dma_start(e2[:], e2_in)
        oms = cpool.tile([S1, S1], F32)
        nc.sync.dma_start(oms[:], oms_in)
        zbias = cpool.tile([128, 1], F32)
        nc.vector.memset(zbias[:], 0.0)

        inpool = ctx.enter_context(tc.tile_pool(name="inp", bufs=2))
        bigpool = ctx.enter_context(tc.tile_pool(name="big", bufs=2))
        mpool = ctx.enter_context(tc.tile_pool(name="mask", bufs=2))
        ptpool = ctx.enter_context(tc.tile_pool(name="pt", bufs=3))
        pvsbp = ctx.enter_context(tc.tile_pool(name="pvsb", bufs=2))
        osbp = ctx.enter_context(tc.tile_pool(name="osb", bufs=3))

        st_ps = ctx.enter_context(tc.tile_pool(name="stps", bufs=2, space="PSUM"))
        pv_ps = ctx.enter_context(tc.tile_pool(name="pvps", bufs=1, space="PSUM"))
        aux_ps = ctx.enter_context(tc.tile_pool(name="auxps", bufs=2, space="PSUM"))

        for p in range(NP):
            # ---- load inputs for this pair ----
            q_sb = inpool.tile([128, NT, D], F32)
            nc.sync.dma_start(q_sb[:], q_in[p].rearrange("(t pp) d -> pp t d", pp=128))
            k_sb = inpool.tile([128, NT, D], F32)
            nc.sync.dma_start(k_sb[:], k_in[p].rearrange("(t pp) d -> pp t d", pp=128))
            v_aug = inpool.tile([128, NT, D + 1], F32R)
            nc.sync.dma_start(
                v_aug[:], v_in[p].rearrange("(t pp) d -> pp t d", pp=128)
            )

            kTA = bigpool.tile([96, S], F32R)
            qC = bigpool.tile([96, S], F32R)
            # constant block-indicator rows of the augmented lhsT
            nc.sync.dma_start(kTA[64:96, :], a_in[:])

            # ---- build qT / kT via PE transposes ----
            for g in range(4):
                tq = aux_ps.tile([64, 512], F32, tag="aux")
                tk = aux_ps.tile([64, 512], F32, tag="aux")
                for c in range(4):
                    t = 4 * g + c
                    nc.tensor.transpose(
                        tq[:, 128 * c : 128 * (c + 1)], q_sb[:, t, :], ident[:]
                    )
                    nc.tensor.transpose(
                        tk[:, 128 * c : 128 * (c + 1)], k_sb[:, t, :], ident[:]
                    )
                nc.vector.tensor_copy(qC[0:64, 512 * g : 512 * (g + 1)], tq[:])
                nc.vector.tensor_copy(kTA[0:64, 512 * g : 512 * (g + 1)], tk[:])

            # ---- block-pooled qaT/kaT  [64, 32] ----
            qa_ps = aux_ps.tile([64, S1], F32, tag="aux")
            ka_ps = aux_ps.tile([64, S1], F32, tag="aux")
            for t in range(NT):
                nc.tensor.matmul(
                    qa_ps[:, 2 * t : 2 * t + 2], lhsT=q_sb[:, t, :], rhs=e2[:],
                    start=True, stop=True,
                )
            for t in range(NT):
                nc.tensor.matmul(
                    ka_ps[:, 2 * t : 2 * t + 2], lhsT=k_sb[:, t, :], rhs=e2[:],
                    start=True, stop=True,
                )
            qa_sb = mpool.tile([64, S1], F32)
            nc.vector.tensor_copy(qa_sb[:], qa_ps[:])
            ka_sb = mpool.tile([64, S1], F32)
            nc.vector.tensor_copy(ka_sb[:], ka_ps[:])

            # ---- block score matrix m and nabla keep rule ----
            m_ps = aux_ps.tile([S1, S1], F32, tag="aux")
            nc.tensor.matmul(m_ps[:], lhsT=qa_sb[:], rhs=ka_sb[:], start=True, stop=True)
            e_sb = mpool.tile([S1, S1], F32)
            nc.scalar.activation(e_sb[:], m_ps[:], AF.Exp, bias=zbias[0:S1, :], scale=SCALE)
            den_sb = mpool.tile([S1, 1], F32)
            nc.vector.reduce_sum(den_sb[:], e_sb[:], axis=mybir.AxisListType.X)
            thr_sb = mpool.tile([S1, 1], F32)
            nc.vector.tensor_scalar_mul(thr_sb[:], den_sb[:], THR)

            # cmp[i, j, j'] = (e[i, j'] <= e[i, j])
            cmp_sb = mpool.tile([S1, S1, S1], F32)
            e_j = e_sb[:, :, None].to_broadcast([S1, S1, S1])
            e_jp = e_sb[:, None, :].to_broadcast([S1, S1, S1])
            nc.vector.tensor_tensor(cmp_sb[:], e_jp, e_j, op=ALU.is_le)
            gm_sb = mpool.tile([S1, S1, S1], F32)
            nc.vector.tensor_tensor(gm_sb[:], cmp_sb[:], e_jp, op=ALU.mult)
            g_sb = mpool.tile([S1, S1], F32)
            nc.vector.reduce_sum(g_sb[:], gm_sb[:], axis=mybir.AxisListType.X)

            # W = NEG * (G < T), then * (1 - sta)
            w_sb = mpool.tile([S1, S1], F32)
            nc.vector.tensor_scalar(
                w_sb[:], g_sb[:], thr_sb[:], NEG, op0=ALU.is_lt, op1=ALU.mult
            )
            w2_sb = mpool.tile([S1, S1], F32)
            nc.vector.tensor_tensor(w2_sb[:], w_sb[:], oms[:], op=ALU.mult)
            wt_sb = mpool.tile([S1, S1], F32)
            nc.vector.transpose(wt_sb[:], w2_sb[:])

            # move WT to partitions 64..95 and expand 64x along free dim -> C2
            stag = mpool.tile([96, S1], F32)
            nc.sync.dma_start(stag[64:96, :], wt_sb[:])
            nc.vector.tensor_copy(
                qC[64:96, :].rearrange("p (r e) -> p r e", e=BLK),
                stag[64:96, :, None].to_broadcast([S1, S1, BLK]),
            )

            # ---- main block-sparse SDPA ----
            for ih in range(2):
                pv = pv_ps.tile([65, 1024], F32)
                for t in range(NT):
                    st = st_ps.tile([128, 1024], F32)
                    for hf in range(2):
                        nc.tensor.matmul(
                            st[:, 512 * hf : 512 * (hf + 1)],
                            lhsT=kTA[:, 128 * t : 128 * (t + 1)],
                            rhs=qC[:, 1024 * ih + 512 * hf : 1024 * ih + 512 * (hf + 1)],
                            start=True, stop=True,
                        )
                    pt = ptpool.tile([128, 1024], F32R)
                    nc.scalar.activation(pt[:], st[:], AF.Exp, bias=zbias[:], scale=SCALE)
                    for hf in range(2):
                        nc.tensor.matmul(
                            pv[:, 512 * hf : 512 * (hf + 1)],
                            lhsT=v_aug[:, t, :],
                            rhs=pt[:, 512 * hf : 512 * (hf + 1)],
                            start=(t == 0), stop=(t == NT - 1),
                        )
                pvs = pvsbp.tile([65, 1024], F32)
                nc.vector.tensor_copy(pvs[:], pv[:])
                for c in range(8):
                    o_ps = aux_ps.tile([128, 65], F32, tag="aux")
                    nc.tensor.transpose(
                        o_ps[:], pvs[:, 128 * c : 128 * (c + 1)], ident[0:65, 0:65]
                    )
                    rd = mpool.tile([128, 1], F32)
                    nc.vector.reciprocal(rd[:], o_ps[:, 64:65])
                    o_sb = osbp.tile([128, D], F32)
                    nc.vector.tensor_scalar(
                        o_sb[:], o_ps[:, 0:D], rd[:], None, op0=ALU.mult
                    )
                    nc.sync.dma_start(
                        o_out[p, 1024 * ih + 128 * c : 1024 * ih + 128 * (c + 1), :],
                        o_sb[:],
                    )

    nc.compile()
    return nc


def get_program():
    if "nc" not in _CACHE:
        _CACHE["nc"] = _build_program()
    return _CACHE["nc"]


def _make_consts(sta_mask):
    e2 = np.zeros((128, 2), np.float32)
    e2[0:64, 0] = 1.0 / BLK
    e2[64:128, 1] = 1.0 / BLK
    a = np.zeros((S1, S), np.float32)
    for r in range(S1):
        a[r, r * BLK : (r + 1) * BLK] = 1.0
    oms = 1.0 - np.asarray(sta_mask).astype(np.float32)
    idn = np.eye(128, dtype=np.float32)
    return e2, a, oms, idn


def make_in_maps(query, key, value, sta_mask):
    query = np.asarray(query, dtype=np.float32)
    key = np.asarray(key, dtype=np.float32)
    value = np.asarray(value, dtype=np.float32)
    e2, a, oms, idn = _make_consts(sta_mask)
    in_maps = []
    for c in range(8):
        b = c // 4
        h0 = 4 * (c % 4)
        in_maps.append(
            {
                "q": np.ascontiguousarray(query[b, :, h0 : h0 + 4, :].transpose(1, 0, 2)),
                "k": np.ascontiguousarray(key[b, :, h0 : h0 + 4, :].transpose(1, 0, 2)),
                "v": np.concatenate([np.ascontiguousarray(value[b, :, h0 : h0 + 4, :].transpose(1, 0, 2)), np.ones((NP, S, 1), np.float32)], axis=-1),
                "e2c": e2,
                "ac": a,
                "oms": oms,
                "idn": idn,
            }
        )
    return in_maps


def gather_output(results):
    out = np.empty((B, S, H * D), np.float32)
    for c in range(8):
        o = results[c]["o"]
        b = c // 4
        h0 = 4 * (c % 4)
        for j in range(NP):
            h = h0 + j
            out[b, :, h * D : (h + 1) * D] = o[j]
    return out


def run(query, key, value, sta_mask, trace=False, **kw):
    nc = get_program()
    in_maps = make_in_maps(query, key, value, sta_mask)
    br = run_bass_kernel_spmd(nc, in_maps, list(range(8)), trace=trace, **kw)
    return gather_output(br.results), br


def kernel(query, key, value, sta_mask):
    out, _ = run(query, key, value, sta_mask)
    return out
